# revision 18
# baseline (speedup 1.0000x reference)
"""Mixture-of-Experts (8 experts, top-2, D=1024, H=2048, T=8192) on 8 trn2 cores.

Strategy: expert-parallel with host-side routing and capacity factor 1.0.
  - Router (tiny: [T,D]@[D,E]) runs on host in float64; top-2 selection was
    verified to match fp32 jax (cpu + neuron) selection for this problem size.
  - Each core owns one expert and computes SwiGLU on the tokens routed to it,
    capped at CAP = T*TOP_K/E = 2048 tokens (capacity factor 1.0).  Spillover
    pairs beyond the cap (<1% of pairs for near-balanced routing) are computed
    exactly on the host during the combine.  Without the cap, SPMD would force
    every core to the hottest expert's padded count (17 x 128-token subtiles
    instead of 16, ~6% more tensor work on every core).
  - Activations flow in transposed (feature-major) layout so the kernel needs
    no on-device transposes:
        h1T = w1 @ xT   (accumulate over D chunks)   [H, C]
        hT  = silu(h1T) * h3T                        [H, C]  (bf16)
        y   = (hT.T chunks) @ w2T                    [C, D]  (tokens on
              partitions so the per-token combine-weight scale is a
              per-partition tensor_scalar op)
  - Host combines: out[t] = y_e1[slot1] + y_e2[slot2] (cw applied on device),
    plus the exact spillover contributions.

Schedule notes (from perfetto traces; PE measured at ~2.0 GHz this chip):
  - Token blocks are [256, 512, ..., 512, 256].  The small first block
    shrinks the gating DMA set (x half + w1 piece 0 = 512 KB) so the first
    real matmul starts ~8.6us instead of ~13us.  N=256 keeps those matmuls
    MM-bound (at N=128 the per-matmul LDWEIGHTS at ~107ns would dominate).
  - Phase order is software-pipelined A0, A1, B0, A2, B1, ... so block 0's
    phase B (which reads all of w2, 4 MB, loaded last) runs ~100us in,
    long after w2 lands.  hpool bufs=2 holds the two live hts tiles.
  - Every input DMA moves a contiguous-per-partition region (>=2KB
    descriptors); block 0's x ships as two half-major chunks so each half
    is one contiguous transfer (descriptor rate, not bandwidth, limits
    small strided transfers).
  - The PE starts at half duty (HAM k=4/8) and is promoted after ~3.4us of
    gap-free matmul activity; 2 dummy matmuls bridge from first possible
    issue (~8.4us, after the NEFF preamble) to the gating DMA receipt.
  - In the final block every non-final subtile is stored as soon as it is
    scaled, and the final subtile is stored in narrowing pieces
    (256/256/256/128/128 wide) so only one small store (plus its HBM write
    receipt) trails the last matmul.
"""

import sys
import types
from contextlib import ExitStack

import ml_dtypes
import numpy as np

import concourse.bass as bass
import concourse.tile as tile
from concourse import bacc, mybir
from concourse.bass_utils import run_bass_kernel_spmd


def install_axon_hooks_shim():
    """The container's antenv stub lacks axon_hooks, which
    run_bass_kernel_spmd imports whenever tracing is requested (including
    via the BASS_TRACE env var). Recreate it and register the NTFF
    profiling hook if the axon PJRT .so is present."""
    try:
        import antenv
    except ImportError:
        return False
    if "antenv.axon_hooks" in sys.modules:
        return sys.modules["antenv.axon_hooks"]._hook is not None
    mod = types.ModuleType("antenv.axon_hooks")
    mod._hook = None
    mod.set_axon_ntff_profile_hook = lambda h: setattr(mod, "_hook", h)
    mod.get_axon_ntff_profile_hook = lambda: mod._hook
    sys.modules["antenv.axon_hooks"] = mod
    antenv.axon_hooks = mod
    try:
        from trn_agent_boot.trn_boot import _ntff_profile_via_ctypes

        mod.set_axon_ntff_profile_hook(
            _ntff_profile_via_ctypes("/opt/axon/libaxon_pjrt.so")
        )
    except Exception:
        pass
    return mod._hook is not None


install_axon_hooks_shim()

E = 8  # experts == cores
D = 1024
H = 2048
TOP_K = 2

# Device capacity per expert (capacity factor 1.0: T * TOP_K / E tokens).
# Spillover pairs beyond this (<1% of pairs for near-balanced routing) are
# computed exactly on the host during the combine, so every core runs the
# same balanced 16-subtile program instead of padding all cores to the
# hottest expert's count.
CAP = 2048

BF16 = mybir.dt.bfloat16
F32 = mybir.dt.float32

_CACHE: dict[int, object] = {}


def _route(x2d: np.ndarray, router_w: np.ndarray):
    """Float64 router. Returns per-expert token lists, per-expert combine
    weights, and for each token its (expert, slot-in-expert-batch) pairs."""
    T = x2d.shape[0]
    logits = x2d.astype(np.float64) @ router_w.astype(np.float64).T  # [T, E]
    order = np.argsort(-logits, axis=1, kind="stable")
    top2 = order[:, :TOP_K]  # [T, 2]
    lt = np.take_along_axis(logits, top2, axis=1)
    m = lt.max(axis=1, keepdims=True)
    ex = np.exp(lt - m)
    cw = (ex / ex.sum(axis=1, keepdims=True)).astype(np.float32)  # [T, 2]

    rows = []  # rows[e]: token ids routed to expert e (ascending)
    cw_e = []  # cw_e[e]: combine weight per routed token
    slot = np.empty((T, TOP_K), np.int64)  # slot[t, k]: row of t in expert batch
    for e in range(E):
        r = np.where((top2[:, 0] == e) | (top2[:, 1] == e))[0]
        k = np.where(top2[r, 0] == e, 0, 1)
        rows.append(r)
        cw_e.append(cw[r, k])
        slot[r, k] = np.arange(len(r))
    return rows, cw_e, top2, slot


# w1/w3 piece sizes in m-chunks (small first so early matmuls start early)
PIECES = (1, 1, 2, 4, 4, 4)

# dummy matmuls issued before the first DMA-gated matmul: bridge from first
# possible PE issue (~8.3us, after the NEFF preamble) to the gating DMA
# receipt (~10.5-11us: barrier 7.2 + 2 DIRECT2D descgens + ~1.4us transfer),
# keeping the HAM activity window gap-free so promotion lands ~3.4us after
# first issue instead of ~3.4us after the gating receipt.
WARMUP_MM = 4


def _blocks_for(C):
    """Token blocks: [256, 256, 512, ..., 512, rem] (rem <= 512).  The two
    small first blocks run phase A interleaved at m-chunk granularity: the
    gating DMA set for the first matmul stays small (x half + w1 piece 0 =
    512 KB) while the weight-piece appetite stays at ~146 GB/s (a solo
    256-token block would consume pieces at ~293 GB/s and starve on HBM)."""
    if C <= 512:
        return [(0, C)]
    blocks = [(0, 256), (256, 256)]
    t0 = 512
    while C - t0 > 512:
        blocks.append((t0, 512))
        t0 += 512
    if C > t0:
        blocks.append((t0, C - t0))
    return blocks


def _build(C: int):
    """Build + compile the per-core Bass program for capacity C (mult of 128).

    All inputs are shipped pre-arranged in SBUF partition-major layout so
    every DMA is ~128 large contiguous descriptors."""
    assert C % 128 == 0
    nsub = C // 128  # token subtiles
    KA = D // 128  # 8 contraction chunks for matmul 1
    KM = H // 128  # 16 contraction chunks for matmul 2
    blocks = _blocks_for(C)
    NB = len(blocks)

    nc = bacc.Bacc("TRN2", target_bir_lowering=False, debug=False)

    # x per token block, partition-major, contiguous per partition.  The
    # first matmuls' gating inputs ship as two "bundles", each one flat
    # contiguous transfer (descriptor generation costs ~730ns per dma_start
    # on the sequencer, so the gating set uses as few transfers as
    # possible): gb1 = [x0 half a<4 | w1 piece 0], gb2 = [x0 half a>=4 |
    # w3 piece 0], flattened per partition.
    assert PIECES[0] == 1
    tb0 = blocks[0][1]
    xs0 = KA // 2
    G1X = xs0 * tb0  # x-half width inside a bundle
    G1W = G1X + KA * 128
    gb1 = nc.declare_dram_parameter("gb1", [128, G1W], BF16, isOutput=False)
    gb2 = nc.declare_dram_parameter("gb2", [128, G1W], BF16, isOutput=False)
    xts_rest = [
        nc.declare_dram_parameter(f"xt{b}", [128, KA, tb], BF16, isOutput=False)
        for b, (t0, tb) in enumerate(blocks)
        if b > 0
    ]
    w1ps = {
        p: nc.declare_dram_parameter(
            f"w1p{p}", [128, KA, sz * 128], BF16, isOutput=False
        )
        for p, sz in enumerate(PIECES)
        if p > 0
    }
    w3ps = {
        p: nc.declare_dram_parameter(
            f"w3p{p}", [128, KA, sz * 128], BF16, isOutput=False
        )
        for p, sz in enumerate(PIECES)
        if p > 0
    }
    w2ps = [
        [
            nc.declare_dram_parameter(
                f"w2p{mh}{dh}", [128, KM // 2, 512], BF16, isOutput=False
            )
            for dh in range(2)
        ]
        for mh in range(2)
    ]
    cwt = nc.declare_dram_parameter("cwt", [128, nsub], F32, isOutput=False)
    y = nc.declare_dram_parameter("y", [C, D], F32, isOutput=True)
    y_t = y.rearrange("(n p) d -> p n d", p=128)  # [128, nsub, D]

    with ExitStack() as ctx:
        tc = ctx.enter_context(tile.TileContext(nc))
        wpool = ctx.enter_context(tc.tile_pool(name="weights", bufs=1))
        xpool = ctx.enter_context(tc.tile_pool(name="x", bufs=3))
        hpool = ctx.enter_context(tc.tile_pool(name="h", bufs=2))
        spool = ctx.enter_context(tc.tile_pool(name="s", bufs=3))
        ypool = ctx.enter_context(tc.tile_pool(name="y", bufs=2))
        ppool = ctx.enter_context(tc.tile_pool(name="psum", bufs=2, space="PSUM"))

        # HAM warmup: dummy matmuls that depend on nothing but two memsets,
        # so they run during the NEFF preamble + gating DMA wait and the PE
        # activity window is gap-free from first issue.
        warmw = wpool.tile([128, 128], BF16, tag="warmw")
        warmx = wpool.tile([128, 512], BF16, tag="warmx")
        nc.vector.memset(warmw[:], 0)
        nc.vector.memset(warmx[:], 0)
        # one shared psum tile: dummies order by PE program order alone (a
        # fresh tile per dummy adds cross-tile WAW semaphore round-trips
        # that were measured to space the dummies ~0.5us apart)
        wp = ppool.tile([128, 512], F32, tag="ph1")
        for _ in range(WARMUP_MM):
            nc.tensor.matmul(wp[:], warmw[:], warmx[:], start=True, stop=True)

        # Gating transfers in consumption order, all on the sync ring: a
        # single ring's FIFO gives the early transfers strict priority.
        # (Splitting them across rings was measured slower: the HW queues
        # round-robin between rings, so late bytes interleave ahead of the
        # gating bytes.)
        gbt1 = wpool.tile([128, G1W], BF16, tag="gb1")
        nc.sync.dma_start(gbt1[:], gb1[:])
        gbt2 = wpool.tile([128, G1W], BF16, tag="gb2")
        nc.sync.dma_start(gbt2[:], gb2[:])

        def xacc0(a):
            g = gbt1 if a < xs0 else gbt2
            return g[:, (a % xs0) * tb0 : (a % xs0 + 1) * tb0]

        xacc = {0: xacc0}
        if NB > 1:
            tb1 = blocks[1][1]
            xts1 = xpool.tile([128, KA, tb1], BF16, tag="xts")
            nc.sync.dma_start(xts1[:], xts_rest[0][:])
            xacc[1] = lambda a: xts1[:, a, :]

        # per m-chunk weight accessors: a -> [128, 128] stationary slice
        w1p = [lambda a: gbt1[:, G1X + a * 128 : G1X + (a + 1) * 128]]
        w3p = [lambda a: gbt2[:, G1X + a * 128 : G1X + (a + 1) * 128]]

        def _wsl(t, i):
            return lambda a: t[:, a, bass.ts(i, 128)]

        for p, sz in enumerate(PIECES):
            if p == 0:
                continue
            t1 = wpool.tile([128, KA, sz * 128], BF16, tag=f"w1s{p}")
            nc.sync.dma_start(t1[:], w1ps[p][:])
            t3 = wpool.tile([128, KA, sz * 128], BF16, tag=f"w3s{p}")
            nc.sync.dma_start(t3[:], w3ps[p][:])
            for i in range(sz):
                w1p.append(_wsl(t1, i))
                w3p.append(_wsl(t3, i))

        w2p = []  # [m-half][d-half] tiles of [128, KM//2, 512]
        for mh in range(2):
            row = []
            for dh in range(2):
                t2 = wpool.tile([128, KM // 2, 512], BF16, tag=f"w2s{mh}{dh}")
                nc.sync.dma_start(t2[:], w2ps[mh][dh][:])
                row.append(t2)
            w2p.append(row)
        cws = wpool.tile([128, nsub], F32, tag="cws")
        nc.sync.dma_start(cws[:], cwt[:])

        def phase_a(group):
            """Phase A for a group of blocks, m-chunk interleaved across the
            group so each weight piece serves all the group's tokens before
            the next piece is needed."""
            xf_g, hts_g = [], []
            for bi in group:
                t0, tb = blocks[bi]
                if bi in xacc:
                    xf = xacc[bi]
                else:
                    xts = xpool.tile(
                        [128, KA, tb], BF16, tag="xts", name=f"xts{bi}"
                    )
                    nc.sync.dma_start(xts[:], xts_rest[bi - 1][:])
                    xf = lambda a, xts=xts: xts[:, a, :]
                xf_g.append(xf)
                hts_g.append(
                    hpool.tile([128, KM, tb], BF16, tag="hts", name=f"hts{bi}")
                )
            for m in range(KM):
                for gi, bi in enumerate(group):
                    tb = blocks[bi][1]
                    xf, hts = xf_g[gi], hts_g[gi]
                    ph1 = ppool.tile([128, tb], F32, tag="ph1")
                    for a in range(KA):
                        nc.tensor.matmul(
                            ph1[:],
                            w1p[m](a),
                            xf(a),
                            start=(a == 0),
                            stop=(a == KA - 1),
                        )
                    ph3 = ppool.tile([128, tb], F32, tag="ph3")
                    for a in range(KA):
                        nc.tensor.matmul(
                            ph3[:],
                            w3p[m](a),
                            xf(a),
                            start=(a == 0),
                            stop=(a == KA - 1),
                        )
                    sil = spool.tile([128, tb], BF16, tag="sil")
                    nc.scalar.activation(
                        sil[:], ph1[:], mybir.ActivationFunctionType.Silu
                    )
                    nc.vector.tensor_mul(hts[:, m, :], sil[:], ph3[:])
            return hts_g

        def phase_b(bi, hts):
            # y = hT.T @ w2T, scaled by cw.  In the final block each subtile
            # is stored as soon as it is scaled; the final subtile is split
            # into narrowing pieces so earlier pieces' scale+store overlap
            # the remaining matmuls and only one small store (plus its HBM
            # write receipt) trails the last matmul.
            t0, tb = blocks[bi]
            nsub_b = tb // 128
            gn0 = t0 // 128
            last_block = bi == NB - 1
            ysb = ypool.tile([128, nsub_b, 1024], F32, tag="ysb")
            for n in range(nsub_b):
                nsl = bass.ts(n, 128)
                gn = gn0 + n  # global subtile index
                final_sub = last_block and n == nsub_b - 1
                if not final_sub:
                    py0 = ppool.tile([128, 512], F32, tag="py0")
                    py1 = ppool.tile([128, 512], F32, tag="py1")
                    for m in range(KM):
                        mh, mr = divmod(m, KM // 2)
                        nc.tensor.matmul(
                            py0[:],
                            hts[:, m, nsl],
                            w2p[mh][0][:, mr, :],
                            start=(m == 0),
                            stop=(m == KM - 1),
                        )
                        nc.tensor.matmul(
                            py1[:],
                            hts[:, m, nsl],
                            w2p[mh][1][:, mr, :],
                            start=(m == 0),
                            stop=(m == KM - 1),
                        )
                    nc.vector.tensor_scalar_mul(
                        ysb[:, n, 0:512], py0[:], cws[:, gn : gn + 1]
                    )
                    nc.vector.tensor_scalar_mul(
                        ysb[:, n, 512:1024], py1[:], cws[:, gn : gn + 1]
                    )
                    if last_block:
                        nc.scalar.dma_start(y_t[:, gn, :], ysb[:, n, :])
                else:
                    pieces = [(0, 0, 256), (0, 256, 256), (1, 0, 256),
                              (1, 256, 128), (1, 384, 128)]
                    for q, (dh, off, wd) in enumerate(pieces):
                        py = ppool.tile([128, wd], F32, tag=f"py{q % 2}")
                        qsl = slice(off, off + wd)
                        for m in range(KM):
                            mh, mr = divmod(m, KM // 2)
                            nc.tensor.matmul(
                                py[:],
                                hts[:, m, nsl],
                                w2p[mh][dh][:, mr, qsl],
                                start=(m == 0),
                                stop=(m == KM - 1),
                            )
                        dsl = slice(dh * 512 + off, dh * 512 + off + wd)
                        nc.vector.tensor_scalar_mul(
                            ysb[:, n, dsl], py[:], cws[:, gn : gn + 1]
                        )
                        # alternate store rings so consecutive pieces'
                        # ~730ns DIRECT2D descriptor-gens overlap (loads
                        # are long done; the sync ring is idle here)
                        ring = nc.sync if q % 2 == 0 else nc.scalar
                        ring.dma_start(y_t[:, gn, dsl], ysb[:, n, dsl])
            if not last_block:
                nc.scalar.dma_start(
                    y_t[:, gn0 : gn0 + nsub_b, :], ysb[:]
                )

        # Software pipeline: A{0,1} interleaved, then B0, A2, B1, A3, B2,
        # ..., B_{NB-1}.  Phase B stays a full block behind phase A so
        # block 0's phase B (first reader of the late-loaded w2) starts
        # ~60us in, after w2 lands; hpool bufs=2 holds the two live hts.
        groups = [[0]] if NB == 1 else [[0, 1]] + [[b] for b in range(2, NB)]
        hts_live = {}
        for bi, hts in zip(groups[0], phase_a(groups[0])):
            hts_live[bi] = hts
        b_next = 0
        for g in groups[1:]:
            phase_b(b_next, hts_live.pop(b_next))
            b_next += 1
            for bi, hts in zip(g, phase_a(g)):
                hts_live[bi] = hts
        while b_next < NB:
            phase_b(b_next, hts_live.pop(b_next))
            b_next += 1

    nc.compile()
    return nc


def _get(C: int):
    if C not in _CACHE:
        _CACHE[C] = _build(C)
    return _CACHE[C]


def _prepare_core_inputs(x2d, w1, w2, w3, rows, cw_e, C):
    bf = ml_dtypes.bfloat16
    nsub = C // 128
    KA, KM = D // 128, H // 128
    blocks = _blocks_for(C)
    xs0 = KA // 2
    in_maps = []
    for e in range(E):
        ce = len(rows[e])
        xt = np.zeros((D, C), bf)
        xt[:, :ce] = x2d[rows[e]].T.astype(bf)
        # partition-major: [128, KA, C]
        xpm = np.ascontiguousarray(xt.reshape(KA, 128, C).transpose(1, 0, 2))

        w1pm = w1[e].T.astype(bf).reshape(KA, 128, H).transpose(1, 0, 2)
        w3pm = w3[e].T.astype(bf).reshape(KA, 128, H).transpose(1, 0, 2)
        w2pm = w2[e].T.astype(bf).reshape(KM, 128, D).transpose(1, 0, 2)

        cwt = np.zeros((C,), np.float32)
        cwt[:ce] = cw_e[e]

        m = {"cwt": np.ascontiguousarray(cwt.reshape(nsub, 128).T)}
        t0_0, tb0 = blocks[0]
        G1X = xs0 * tb0
        gb = np.empty((2, 128, G1X + KA * 128), bf)
        gb[0, :, :G1X] = xpm[:, 0:xs0, 0:tb0].reshape(128, G1X)
        gb[1, :, :G1X] = xpm[:, xs0:KA, 0:tb0].reshape(128, G1X)
        gb[0, :, G1X:] = w1pm[:, :, 0:128].reshape(128, KA * 128)
        gb[1, :, G1X:] = w3pm[:, :, 0:128].reshape(128, KA * 128)
        m["gb1"] = np.ascontiguousarray(gb[0])
        m["gb2"] = np.ascontiguousarray(gb[1])
        for b, (t0, tb) in enumerate(blocks):
            if b == 0:
                continue
            m[f"xt{b}"] = np.ascontiguousarray(xpm[:, :, t0 : t0 + tb])
        m0 = 0
        for p, sz in enumerate(PIECES):
            if p > 0:
                hs = slice(m0 * 128, (m0 + sz) * 128)
                m[f"w1p{p}"] = np.ascontiguousarray(w1pm[:, :, hs])
                m[f"w3p{p}"] = np.ascontiguousarray(w3pm[:, :, hs])
            m0 += sz
        for mh in range(2):
            msl = slice(mh * (KM // 2), (mh + 1) * (KM // 2))
            for dh in range(2):
                m[f"w2p{mh}{dh}"] = np.ascontiguousarray(
                    w2pm[:, msl, dh * 512 : (dh + 1) * 512]
                )
        in_maps.append(m)
    return in_maps


def run(inputs: dict, trace: bool = False, trace_cores=None):
    """Core implementation; returns (output, BassKernelResults)."""
    x = np.asarray(inputs["x"])
    router_w = np.asarray(inputs["router_w"], np.float32)
    w1 = np.asarray(inputs["w1"], np.float32)
    w2 = np.asarray(inputs["w2"], np.float32)
    w3 = np.asarray(inputs["w3"], np.float32)

    B, S, _ = x.shape
    assert x.shape[-1] == D and router_w.shape == (E, D), (x.shape, router_w.shape)
    assert w1.shape == (E, H, D) and w3.shape == (E, H, D) and w2.shape == (E, D, H)
    x2d = np.ascontiguousarray(x.reshape(-1, D).astype(np.float32))
    T = x2d.shape[0]

    rows, cw_e, top2, slot = _route(x2d, router_w)
    rows_d = [r[:CAP] for r in rows]
    cw_d = [c[:CAP] for c in cw_e]
    spill = [
        (e, rows[e][CAP:], cw_e[e][CAP:]) for e in range(E) if len(rows[e]) > CAP
    ]
    cmax = max(len(r) for r in rows_d)
    C = max(128, int(np.ceil(cmax / 128) * 128))

    nc = _get(C)
    in_maps = _prepare_core_inputs(x2d, w1, w2, w3, rows_d, cw_d, C)
    res = run_bass_kernel_spmd(
        nc,
        in_maps,
        list(range(E)),
        trace=trace,
        trace_cores=trace_cores,
    )

    Y = np.stack([res.results[e]["y"] for e in range(E)])  # [E, C, D] f32
    Yf = Y.reshape(E * C, D)
    valid = slot < C  # [T, 2]; spilled pairs resolved on host below
    fi = top2.astype(np.int64) * C + np.minimum(slot, C - 1)
    out = Yf[fi[:, 0]] * valid[:, 0:1] + Yf[fi[:, 1]] * valid[:, 1:2]

    for e, r, c in spill:
        xo = x2d[r]
        h1 = xo @ w1[e].T
        h = (h1 / (1.0 + np.exp(-h1))) * (xo @ w3[e].T)
        out[r] += (h @ w2[e].T) * c[:, None]
    return out.reshape(B, S, D).astype(x.dtype), res


def kernel(**inputs) -> np.ndarray:
    out, _ = run(inputs, trace=False)
    return out


# revision 19
# speedup vs baseline: 1.0033x; 1.0033x over previous
"""Mixture-of-Experts (8 experts, top-2, D=1024, H=2048, T=8192) on 8 trn2 cores.

Strategy: expert-parallel with host-side routing and capacity factor 1.0.
  - Router (tiny: [T,D]@[D,E]) runs on host in float64; top-2 selection was
    verified to match fp32 jax (cpu + neuron) selection for this problem size.
  - Each core owns one expert and computes SwiGLU on the tokens routed to it,
    capped at CAP = T*TOP_K/E = 2048 tokens (capacity factor 1.0).  Spillover
    pairs beyond the cap (<1% of pairs for near-balanced routing) are computed
    exactly on the host during the combine.  Without the cap, SPMD would force
    every core to the hottest expert's padded count (17 x 128-token subtiles
    instead of 16, ~6% more tensor work on every core).
  - Activations flow in transposed (feature-major) layout so the kernel needs
    no on-device transposes:
        h1T = w1 @ xT   (accumulate over D chunks)   [H, C]
        hT  = silu(h1T) * h3T                        [H, C]  (bf16)
        y   = (hT.T chunks) @ w2T                    [C, D]  (tokens on
              partitions so the per-token combine-weight scale is a
              per-partition tensor_scalar op)
  - Host combines: out[t] = y_e1[slot1] + y_e2[slot2] (cw applied on device),
    plus the exact spillover contributions.

Schedule notes (from perfetto traces; PE measured at ~2.0 GHz this chip):
  - Token blocks are [256, 512, ..., 512, 256].  The small first block
    shrinks the gating DMA set (x half + w1 piece 0 = 512 KB) so the first
    real matmul starts ~8.6us instead of ~13us.  N=256 keeps those matmuls
    MM-bound (at N=128 the per-matmul LDWEIGHTS at ~107ns would dominate).
  - Phase order is software-pipelined A0, A1, B0, A2, B1, ... so block 0's
    phase B (which reads all of w2, 4 MB, loaded last) runs ~100us in,
    long after w2 lands.  hpool bufs=2 holds the two live hts tiles.
  - Every input DMA moves a contiguous-per-partition region (>=2KB
    descriptors); block 0's x ships as two half-major chunks so each half
    is one contiguous transfer (descriptor rate, not bandwidth, limits
    small strided transfers).
  - The PE starts at half duty (HAM k=4/8) and is promoted after ~3.4us of
    gap-free matmul activity; 2 dummy matmuls bridge from first possible
    issue (~8.4us, after the NEFF preamble) to the gating DMA receipt.
  - In the final block every non-final subtile is stored as soon as it is
    scaled, and the final subtile is stored in narrowing pieces
    (256/256/256/128/128 wide) so only one small store (plus its HBM write
    receipt) trails the last matmul.
"""

import sys
import types
from contextlib import ExitStack

import ml_dtypes
import numpy as np

import concourse.bass as bass
import concourse.tile as tile
from concourse import bacc, mybir
from concourse.bass_utils import run_bass_kernel_spmd


def install_axon_hooks_shim():
    """The container's antenv stub lacks axon_hooks, which
    run_bass_kernel_spmd imports whenever tracing is requested (including
    via the BASS_TRACE env var). Recreate it and register the NTFF
    profiling hook if the axon PJRT .so is present."""
    try:
        import antenv
    except ImportError:
        return False
    if "antenv.axon_hooks" in sys.modules:
        return sys.modules["antenv.axon_hooks"]._hook is not None
    mod = types.ModuleType("antenv.axon_hooks")
    mod._hook = None
    mod.set_axon_ntff_profile_hook = lambda h: setattr(mod, "_hook", h)
    mod.get_axon_ntff_profile_hook = lambda: mod._hook
    sys.modules["antenv.axon_hooks"] = mod
    antenv.axon_hooks = mod
    try:
        from trn_agent_boot.trn_boot import _ntff_profile_via_ctypes

        mod.set_axon_ntff_profile_hook(
            _ntff_profile_via_ctypes("/opt/axon/libaxon_pjrt.so")
        )
    except Exception:
        pass
    return mod._hook is not None


install_axon_hooks_shim()

E = 8  # experts == cores
D = 1024
H = 2048
TOP_K = 2

# Device capacity per expert (capacity factor 1.0: T * TOP_K / E tokens).
# Spillover pairs beyond this (<1% of pairs for near-balanced routing) are
# computed exactly on the host during the combine, so every core runs the
# same balanced 16-subtile program instead of padding all cores to the
# hottest expert's count.
CAP = 2048

BF16 = mybir.dt.bfloat16
F32 = mybir.dt.float32

_CACHE: dict[int, object] = {}


def _route(x2d: np.ndarray, router_w: np.ndarray):
    """Float64 router. Returns per-expert token lists, per-expert combine
    weights, and for each token its (expert, slot-in-expert-batch) pairs."""
    T = x2d.shape[0]
    logits = x2d.astype(np.float64) @ router_w.astype(np.float64).T  # [T, E]
    order = np.argsort(-logits, axis=1, kind="stable")
    top2 = order[:, :TOP_K]  # [T, 2]
    lt = np.take_along_axis(logits, top2, axis=1)
    m = lt.max(axis=1, keepdims=True)
    ex = np.exp(lt - m)
    cw = (ex / ex.sum(axis=1, keepdims=True)).astype(np.float32)  # [T, 2]

    rows = []  # rows[e]: token ids routed to expert e (ascending)
    cw_e = []  # cw_e[e]: combine weight per routed token
    slot = np.empty((T, TOP_K), np.int64)  # slot[t, k]: row of t in expert batch
    for e in range(E):
        r = np.where((top2[:, 0] == e) | (top2[:, 1] == e))[0]
        k = np.where(top2[r, 0] == e, 0, 1)
        rows.append(r)
        cw_e.append(cw[r, k])
        slot[r, k] = np.arange(len(r))
    return rows, cw_e, top2, slot


# w1/w3 piece sizes in m-chunks (small first so early matmuls start early)
PIECES = (1, 1, 2, 4, 4, 4)

# dummy matmuls issued before the first DMA-gated matmul: bridge from first
# possible PE issue (~8.3us, after the NEFF preamble) to the gating DMA
# receipt (~10.5-11us: barrier 7.2 + 2 DIRECT2D descgens + ~1.4us transfer),
# keeping the HAM activity window gap-free so promotion lands ~3.4us after
# first issue instead of ~3.4us after the gating receipt.
WARMUP_MM = 4


def _blocks_for(C):
    """Token blocks: [256, 256, 512, ..., 512, rem] (rem <= 512).  The two
    small first blocks run phase A interleaved at m-chunk granularity: the
    gating DMA set for the first matmul stays small (x half + w1 piece 0 =
    512 KB) while the weight-piece appetite stays at ~146 GB/s (a solo
    256-token block would consume pieces at ~293 GB/s and starve on HBM)."""
    if C <= 512:
        return [(0, C)]
    blocks = [(0, 256), (256, 256)]
    t0 = 512
    while C - t0 > 512:
        blocks.append((t0, 512))
        t0 += 512
    if C > t0:
        blocks.append((t0, C - t0))
    return blocks


def _build(C: int):
    """Build + compile the per-core Bass program for capacity C (mult of 128).

    All inputs are shipped pre-arranged in SBUF partition-major layout so
    every DMA is ~128 large contiguous descriptors."""
    assert C % 128 == 0
    nsub = C // 128  # token subtiles
    KA = D // 128  # 8 contraction chunks for matmul 1
    KM = H // 128  # 16 contraction chunks for matmul 2
    blocks = _blocks_for(C)
    NB = len(blocks)

    nc = bacc.Bacc("TRN2", target_bir_lowering=False, debug=False)

    # x per token block, partition-major, contiguous per partition.  The
    # first matmuls' gating inputs ship as two "bundles", each one flat
    # contiguous transfer (descriptor generation costs ~730ns per dma_start
    # on the sequencer, so the gating set uses as few transfers as
    # possible): gb1 = [x0 half a<4 | w1 piece 0], gb2 = [x0 half a>=4 |
    # w3 piece 0], flattened per partition.
    assert PIECES[0] == 1
    tb0 = blocks[0][1]
    xs0 = KA // 2
    G1X = xs0 * tb0  # x-half width inside a bundle
    G1W = G1X + KA * 128
    gb1 = nc.declare_dram_parameter("gb1", [128, G1W], BF16, isOutput=False)
    gb2 = nc.declare_dram_parameter("gb2", [128, G1W], BF16, isOutput=False)
    xts_rest = [
        nc.declare_dram_parameter(f"xt{b}", [128, KA, tb], BF16, isOutput=False)
        for b, (t0, tb) in enumerate(blocks)
        if b > 0
    ]
    w1ps = {
        p: nc.declare_dram_parameter(
            f"w1p{p}", [128, KA, sz * 128], BF16, isOutput=False
        )
        for p, sz in enumerate(PIECES)
        if p > 0
    }
    w3ps = {
        p: nc.declare_dram_parameter(
            f"w3p{p}", [128, KA, sz * 128], BF16, isOutput=False
        )
        for p, sz in enumerate(PIECES)
        if p > 0
    }
    w2ps = [
        [
            nc.declare_dram_parameter(
                f"w2p{mh}{dh}", [128, KM // 2, 512], BF16, isOutput=False
            )
            for dh in range(2)
        ]
        for mh in range(2)
    ]
    cwt = nc.declare_dram_parameter("cwt", [128, nsub], F32, isOutput=False)
    y = nc.declare_dram_parameter("y", [C, D], F32, isOutput=True)
    y_t = y.rearrange("(n p) d -> p n d", p=128)  # [128, nsub, D]

    with ExitStack() as ctx:
        tc = ctx.enter_context(tile.TileContext(nc))
        wpool = ctx.enter_context(tc.tile_pool(name="weights", bufs=1))
        xpool = ctx.enter_context(tc.tile_pool(name="x", bufs=3))
        hpool = ctx.enter_context(tc.tile_pool(name="h", bufs=2))
        spool = ctx.enter_context(tc.tile_pool(name="s", bufs=3))
        ypool = ctx.enter_context(tc.tile_pool(name="y", bufs=2))
        ppool = ctx.enter_context(tc.tile_pool(name="psum", bufs=2, space="PSUM"))

        # HAM warmup: dummy matmuls that depend on nothing but two memsets,
        # so they run during the NEFF preamble + gating DMA wait and the PE
        # activity window is gap-free from first issue.
        warmw = wpool.tile([128, 128], BF16, tag="warmw")
        warmx = wpool.tile([128, 512], BF16, tag="warmx")
        nc.vector.memset(warmw[:], 0)
        nc.vector.memset(warmx[:], 0)
        # one shared psum tile: dummies order by PE program order alone (a
        # fresh tile per dummy adds cross-tile WAW semaphore round-trips
        # that were measured to space the dummies ~0.5us apart)
        wp = ppool.tile([128, 512], F32, tag="ph1")
        for _ in range(WARMUP_MM):
            nc.tensor.matmul(wp[:], warmw[:], warmx[:], start=True, stop=True)

        # Gating transfers in consumption order, all on the sync ring: a
        # single ring's FIFO gives the early transfers strict priority.
        # (Splitting them across rings was measured slower: the HW queues
        # round-robin between rings, so late bytes interleave ahead of the
        # gating bytes.)
        gbt1 = wpool.tile([128, G1W], BF16, tag="gb1")
        nc.sync.dma_start(gbt1[:], gb1[:])
        gbt2 = wpool.tile([128, G1W], BF16, tag="gb2")
        nc.sync.dma_start(gbt2[:], gb2[:])

        def xacc0(a):
            g = gbt1 if a < xs0 else gbt2
            return g[:, (a % xs0) * tb0 : (a % xs0 + 1) * tb0]

        xacc = {0: xacc0}
        if NB > 1:
            tb1 = blocks[1][1]
            xts1 = xpool.tile([128, KA, tb1], BF16, tag="xts")
            nc.sync.dma_start(xts1[:], xts_rest[0][:])
            xacc[1] = lambda a: xts1[:, a, :]

        # per m-chunk weight accessors: a -> [128, 128] stationary slice
        w1p = [lambda a: gbt1[:, G1X + a * 128 : G1X + (a + 1) * 128]]
        w3p = [lambda a: gbt2[:, G1X + a * 128 : G1X + (a + 1) * 128]]

        def _wsl(t, i):
            return lambda a: t[:, a, bass.ts(i, 128)]

        for p, sz in enumerate(PIECES):
            if p == 0:
                continue
            t1 = wpool.tile([128, KA, sz * 128], BF16, tag=f"w1s{p}")
            nc.sync.dma_start(t1[:], w1ps[p][:])
            t3 = wpool.tile([128, KA, sz * 128], BF16, tag=f"w3s{p}")
            nc.sync.dma_start(t3[:], w3ps[p][:])
            for i in range(sz):
                w1p.append(_wsl(t1, i))
                w3p.append(_wsl(t3, i))

        w2p = []  # [m-half][d-half] tiles of [128, KM//2, 512]
        for mh in range(2):
            row = []
            for dh in range(2):
                t2 = wpool.tile([128, KM // 2, 512], BF16, tag=f"w2s{mh}{dh}")
                nc.sync.dma_start(t2[:], w2ps[mh][dh][:])
                row.append(t2)
            w2p.append(row)
        cws = wpool.tile([128, nsub], F32, tag="cws")
        nc.sync.dma_start(cws[:], cwt[:])

        def phase_a(group):
            """Phase A for a group of blocks, m-chunk interleaved across the
            group so each weight piece serves all the group's tokens before
            the next piece is needed."""
            xf_g, hts_g = [], []
            for bi in group:
                t0, tb = blocks[bi]
                if bi in xacc:
                    xf = xacc[bi]
                else:
                    xts = xpool.tile(
                        [128, KA, tb], BF16, tag="xts", name=f"xts{bi}"
                    )
                    nc.sync.dma_start(xts[:], xts_rest[bi - 1][:])
                    xf = lambda a, xts=xts: xts[:, a, :]
                xf_g.append(xf)
                hts_g.append(
                    hpool.tile([128, KM, tb], BF16, tag="hts", name=f"hts{bi}")
                )
            for m in range(KM):
                for gi, bi in enumerate(group):
                    tb = blocks[bi][1]
                    xf, hts = xf_g[gi], hts_g[gi]
                    ph1 = ppool.tile([128, tb], F32, tag="ph1")
                    for a in range(KA):
                        nc.tensor.matmul(
                            ph1[:],
                            w1p[m](a),
                            xf(a),
                            start=(a == 0),
                            stop=(a == KA - 1),
                        )
                    ph3 = ppool.tile([128, tb], F32, tag="ph3")
                    for a in range(KA):
                        nc.tensor.matmul(
                            ph3[:],
                            w3p[m](a),
                            xf(a),
                            start=(a == 0),
                            stop=(a == KA - 1),
                        )
                    sil = spool.tile([128, tb], BF16, tag="sil")
                    nc.scalar.activation(
                        sil[:], ph1[:], mybir.ActivationFunctionType.Silu
                    )
                    nc.vector.tensor_mul(hts[:, m, :], sil[:], ph3[:])
            return hts_g

        def phase_b(bi, hts):
            # y = hT.T @ w2T, scaled by cw.  In the final block each subtile
            # is stored as soon as it is scaled; the final subtile is split
            # into narrowing pieces so earlier pieces' scale+store overlap
            # the remaining matmuls and only one small store (plus its HBM
            # write receipt) trails the last matmul.
            t0, tb = blocks[bi]
            nsub_b = tb // 128
            gn0 = t0 // 128
            last_block = bi == NB - 1
            ysb = ypool.tile([128, nsub_b, 1024], F32, tag="ysb")
            for n in range(nsub_b):
                nsl = bass.ts(n, 128)
                gn = gn0 + n  # global subtile index
                final_sub = last_block and n == nsub_b - 1
                if not final_sub:
                    py0 = ppool.tile([128, 512], F32, tag="py0")
                    py1 = ppool.tile([128, 512], F32, tag="py1")
                    for m in range(KM):
                        mh, mr = divmod(m, KM // 2)
                        nc.tensor.matmul(
                            py0[:],
                            hts[:, m, nsl],
                            w2p[mh][0][:, mr, :],
                            start=(m == 0),
                            stop=(m == KM - 1),
                        )
                        nc.tensor.matmul(
                            py1[:],
                            hts[:, m, nsl],
                            w2p[mh][1][:, mr, :],
                            start=(m == 0),
                            stop=(m == KM - 1),
                        )
                    nc.vector.tensor_scalar_mul(
                        ysb[:, n, 0:512], py0[:], cws[:, gn : gn + 1]
                    )
                    nc.vector.tensor_scalar_mul(
                        ysb[:, n, 512:1024], py1[:], cws[:, gn : gn + 1]
                    )
                    if last_block:
                        nc.scalar.dma_start(y_t[:, gn, :], ysb[:, n, :])
                else:
                    pieces = [(0, 0, 256), (0, 256, 256), (1, 0, 256),
                              (1, 256, 128), (1, 384, 128)]
                    for q, (dh, off, wd) in enumerate(pieces):
                        py = ppool.tile([128, wd], F32, tag=f"py{q % 2}")
                        qsl = slice(off, off + wd)
                        for m in range(KM):
                            mh, mr = divmod(m, KM // 2)
                            nc.tensor.matmul(
                                py[:],
                                hts[:, m, nsl],
                                w2p[mh][dh][:, mr, qsl],
                                start=(m == 0),
                                stop=(m == KM - 1),
                            )
                        dsl = slice(dh * 512 + off, dh * 512 + off + wd)
                        nc.vector.tensor_scalar_mul(
                            ysb[:, n, dsl], py[:], cws[:, gn : gn + 1]
                        )
                        nc.scalar.dma_start(y_t[:, gn, dsl], ysb[:, n, dsl])
            if not last_block:
                nc.scalar.dma_start(
                    y_t[:, gn0 : gn0 + nsub_b, :], ysb[:]
                )

        # Software pipeline: A{0,1} interleaved, then B0, A2, B1, A3, B2,
        # ..., B_{NB-1}.  Phase B stays a full block behind phase A so
        # block 0's phase B (first reader of the late-loaded w2) starts
        # ~60us in, after w2 lands; hpool bufs=2 holds the two live hts.
        groups = [[0]] if NB == 1 else [[0, 1]] + [[b] for b in range(2, NB)]
        hts_live = {}
        for bi, hts in zip(groups[0], phase_a(groups[0])):
            hts_live[bi] = hts
        b_next = 0
        for g in groups[1:]:
            phase_b(b_next, hts_live.pop(b_next))
            b_next += 1
            for bi, hts in zip(g, phase_a(g)):
                hts_live[bi] = hts
        while b_next < NB:
            phase_b(b_next, hts_live.pop(b_next))
            b_next += 1

    nc.compile()
    return nc


def _get(C: int):
    if C not in _CACHE:
        _CACHE[C] = _build(C)
    return _CACHE[C]


def _prepare_core_inputs(x2d, w1, w2, w3, rows, cw_e, C):
    bf = ml_dtypes.bfloat16
    nsub = C // 128
    KA, KM = D // 128, H // 128
    blocks = _blocks_for(C)
    xs0 = KA // 2
    in_maps = []
    for e in range(E):
        ce = len(rows[e])
        xt = np.zeros((D, C), bf)
        xt[:, :ce] = x2d[rows[e]].T.astype(bf)
        # partition-major: [128, KA, C]
        xpm = np.ascontiguousarray(xt.reshape(KA, 128, C).transpose(1, 0, 2))

        w1pm = w1[e].T.astype(bf).reshape(KA, 128, H).transpose(1, 0, 2)
        w3pm = w3[e].T.astype(bf).reshape(KA, 128, H).transpose(1, 0, 2)
        w2pm = w2[e].T.astype(bf).reshape(KM, 128, D).transpose(1, 0, 2)

        cwt = np.zeros((C,), np.float32)
        cwt[:ce] = cw_e[e]

        m = {"cwt": np.ascontiguousarray(cwt.reshape(nsub, 128).T)}
        t0_0, tb0 = blocks[0]
        G1X = xs0 * tb0
        gb = np.empty((2, 128, G1X + KA * 128), bf)
        gb[0, :, :G1X] = xpm[:, 0:xs0, 0:tb0].reshape(128, G1X)
        gb[1, :, :G1X] = xpm[:, xs0:KA, 0:tb0].reshape(128, G1X)
        gb[0, :, G1X:] = w1pm[:, :, 0:128].reshape(128, KA * 128)
        gb[1, :, G1X:] = w3pm[:, :, 0:128].reshape(128, KA * 128)
        m["gb1"] = np.ascontiguousarray(gb[0])
        m["gb2"] = np.ascontiguousarray(gb[1])
        for b, (t0, tb) in enumerate(blocks):
            if b == 0:
                continue
            m[f"xt{b}"] = np.ascontiguousarray(xpm[:, :, t0 : t0 + tb])
        m0 = 0
        for p, sz in enumerate(PIECES):
            if p > 0:
                hs = slice(m0 * 128, (m0 + sz) * 128)
                m[f"w1p{p}"] = np.ascontiguousarray(w1pm[:, :, hs])
                m[f"w3p{p}"] = np.ascontiguousarray(w3pm[:, :, hs])
            m0 += sz
        for mh in range(2):
            msl = slice(mh * (KM // 2), (mh + 1) * (KM // 2))
            for dh in range(2):
                m[f"w2p{mh}{dh}"] = np.ascontiguousarray(
                    w2pm[:, msl, dh * 512 : (dh + 1) * 512]
                )
        in_maps.append(m)
    return in_maps


def run(inputs: dict, trace: bool = False, trace_cores=None):
    """Core implementation; returns (output, BassKernelResults)."""
    x = np.asarray(inputs["x"])
    router_w = np.asarray(inputs["router_w"], np.float32)
    w1 = np.asarray(inputs["w1"], np.float32)
    w2 = np.asarray(inputs["w2"], np.float32)
    w3 = np.asarray(inputs["w3"], np.float32)

    B, S, _ = x.shape
    assert x.shape[-1] == D and router_w.shape == (E, D), (x.shape, router_w.shape)
    assert w1.shape == (E, H, D) and w3.shape == (E, H, D) and w2.shape == (E, D, H)
    x2d = np.ascontiguousarray(x.reshape(-1, D).astype(np.float32))
    T = x2d.shape[0]

    rows, cw_e, top2, slot = _route(x2d, router_w)
    rows_d = [r[:CAP] for r in rows]
    cw_d = [c[:CAP] for c in cw_e]
    spill = [
        (e, rows[e][CAP:], cw_e[e][CAP:]) for e in range(E) if len(rows[e]) > CAP
    ]
    cmax = max(len(r) for r in rows_d)
    C = max(128, int(np.ceil(cmax / 128) * 128))

    nc = _get(C)
    in_maps = _prepare_core_inputs(x2d, w1, w2, w3, rows_d, cw_d, C)
    res = run_bass_kernel_spmd(
        nc,
        in_maps,
        list(range(E)),
        trace=trace,
        trace_cores=trace_cores,
    )

    Y = np.stack([res.results[e]["y"] for e in range(E)])  # [E, C, D] f32
    Yf = Y.reshape(E * C, D)
    valid = slot < C  # [T, 2]; spilled pairs resolved on host below
    fi = top2.astype(np.int64) * C + np.minimum(slot, C - 1)
    out = Yf[fi[:, 0]] * valid[:, 0:1] + Yf[fi[:, 1]] * valid[:, 1:2]

    for e, r, c in spill:
        xo = x2d[r]
        h1 = xo @ w1[e].T
        h = (h1 / (1.0 + np.exp(-h1))) * (xo @ w3[e].T)
        out[r] += (h @ w2[e].T) * c[:, None]
    return out.reshape(B, S, D).astype(x.dtype), res


def kernel(**inputs) -> np.ndarray:
    out, _ = run(inputs, trace=False)
    return out


# revision 22
# speedup vs baseline: 1.0086x; 1.0052x over previous
"""Mixture-of-Experts (8 experts, top-2, D=1024, H=2048, T=8192) on 8 trn2 cores.

Strategy: expert-parallel with host-side routing and capacity factor 1.0.
  - Router (tiny: [T,D]@[D,E]) runs on host in float64; top-2 selection was
    verified to match fp32 jax (cpu + neuron) selection for this problem size.
  - Each core owns one expert and computes SwiGLU on the tokens routed to it,
    capped at CAP = T*TOP_K/E = 2048 tokens (capacity factor 1.0).  Spillover
    pairs beyond the cap (<1% of pairs for near-balanced routing) are computed
    exactly on the host during the combine.  Without the cap, SPMD would force
    every core to the hottest expert's padded count (17 x 128-token subtiles
    instead of 16, ~6% more tensor work on every core).
  - Activations flow in transposed (feature-major) layout so the kernel needs
    no on-device transposes:
        h1T = w1 @ xT   (accumulate over D chunks)   [H, C]
        hT  = silu(h1T) * h3T                        [H, C]  (bf16)
        y   = (hT.T chunks) @ w2T                    [C, D]  (tokens on
              partitions so the per-token combine-weight scale is a
              per-partition tensor_scalar op)
  - Host combines: out[t] = y_e1[slot1] + y_e2[slot2] (cw applied on device),
    plus the exact spillover contributions.

Schedule notes (from perfetto traces; PE measured at ~2.0 GHz this chip):
  - Token blocks are [256, 512, ..., 512, 256].  The small first block
    shrinks the gating DMA set (x half + w1 piece 0 = 512 KB) so the first
    real matmul starts ~8.6us instead of ~13us.  N=256 keeps those matmuls
    MM-bound (at N=128 the per-matmul LDWEIGHTS at ~107ns would dominate).
  - Phase order is software-pipelined A0, A1, B0, A2, B1, ... so block 0's
    phase B (which reads all of w2, 4 MB, loaded last) runs ~100us in,
    long after w2 lands.  hpool bufs=2 holds the two live hts tiles.
  - Every input DMA moves a contiguous-per-partition region (>=2KB
    descriptors); block 0's x ships as two half-major chunks so each half
    is one contiguous transfer (descriptor rate, not bandwidth, limits
    small strided transfers).
  - The PE starts at half duty (HAM k=4/8) and is promoted after ~3.4us of
    gap-free matmul activity; 2 dummy matmuls bridge from first possible
    issue (~8.4us, after the NEFF preamble) to the gating DMA receipt.
  - In the final block every non-final subtile is stored as soon as it is
    scaled, and the final subtile is stored in narrowing pieces
    (256/256/256/128/128 wide) so only one small store (plus its HBM write
    receipt) trails the last matmul.
"""

import sys
import types
from contextlib import ExitStack

import ml_dtypes
import numpy as np

import concourse.bass as bass
import concourse.tile as tile
from concourse import bacc, mybir
from concourse.bass_utils import run_bass_kernel_spmd


def install_axon_hooks_shim():
    """The container's antenv stub lacks axon_hooks, which
    run_bass_kernel_spmd imports whenever tracing is requested (including
    via the BASS_TRACE env var). Recreate it and register the NTFF
    profiling hook if the axon PJRT .so is present."""
    try:
        import antenv
    except ImportError:
        return False
    if "antenv.axon_hooks" in sys.modules:
        return sys.modules["antenv.axon_hooks"]._hook is not None
    mod = types.ModuleType("antenv.axon_hooks")
    mod._hook = None
    mod.set_axon_ntff_profile_hook = lambda h: setattr(mod, "_hook", h)
    mod.get_axon_ntff_profile_hook = lambda: mod._hook
    sys.modules["antenv.axon_hooks"] = mod
    antenv.axon_hooks = mod
    try:
        from trn_agent_boot.trn_boot import _ntff_profile_via_ctypes

        mod.set_axon_ntff_profile_hook(
            _ntff_profile_via_ctypes("/opt/axon/libaxon_pjrt.so")
        )
    except Exception:
        pass
    return mod._hook is not None


install_axon_hooks_shim()

E = 8  # experts == cores
D = 1024
H = 2048
TOP_K = 2

# Device capacity per expert (capacity factor 1.0: T * TOP_K / E tokens).
# Spillover pairs beyond this (<1% of pairs for near-balanced routing) are
# computed exactly on the host during the combine, so every core runs the
# same balanced 16-subtile program instead of padding all cores to the
# hottest expert's count.
CAP = 2048

BF16 = mybir.dt.bfloat16
F32 = mybir.dt.float32

_CACHE: dict[int, object] = {}


def _route(x2d: np.ndarray, router_w: np.ndarray):
    """Float64 router. Returns per-expert token lists, per-expert combine
    weights, and for each token its (expert, slot-in-expert-batch) pairs."""
    T = x2d.shape[0]
    logits = x2d.astype(np.float64) @ router_w.astype(np.float64).T  # [T, E]
    order = np.argsort(-logits, axis=1, kind="stable")
    top2 = order[:, :TOP_K]  # [T, 2]
    lt = np.take_along_axis(logits, top2, axis=1)
    m = lt.max(axis=1, keepdims=True)
    ex = np.exp(lt - m)
    cw = (ex / ex.sum(axis=1, keepdims=True)).astype(np.float32)  # [T, 2]

    rows = []  # rows[e]: token ids routed to expert e (ascending)
    cw_e = []  # cw_e[e]: combine weight per routed token
    slot = np.empty((T, TOP_K), np.int64)  # slot[t, k]: row of t in expert batch
    for e in range(E):
        r = np.where((top2[:, 0] == e) | (top2[:, 1] == e))[0]
        k = np.where(top2[r, 0] == e, 0, 1)
        rows.append(r)
        cw_e.append(cw[r, k])
        slot[r, k] = np.arange(len(r))
    return rows, cw_e, top2, slot


# w1/w3 piece sizes in m-chunks (small first so early matmuls start early)
PIECES = (1, 1, 2, 4, 4, 4)

# dummy matmuls issued before the first DMA-gated matmul: bridge from first
# possible PE issue (~8.3us, after the NEFF preamble) to the gating DMA
# receipt (~10.5-11us: barrier 7.2 + 2 DIRECT2D descgens + ~1.4us transfer),
# keeping the HAM activity window gap-free so promotion lands ~3.4us after
# first issue instead of ~3.4us after the gating receipt.
WARMUP_MM = 7


def _blocks_for(C):
    """Token blocks: [256, 256, 512, ..., 512, rem] (rem <= 512).  The two
    small first blocks run phase A interleaved at m-chunk granularity: the
    gating DMA set for the first matmul stays small (x half + w1 piece 0 =
    512 KB) while the weight-piece appetite stays at ~146 GB/s (a solo
    256-token block would consume pieces at ~293 GB/s and starve on HBM)."""
    if C <= 512:
        return [(0, C)]
    blocks = [(0, 256), (256, 256)]
    t0 = 512
    while C - t0 > 512:
        blocks.append((t0, 512))
        t0 += 512
    if C > t0:
        blocks.append((t0, C - t0))
    return blocks


def _build(C: int):
    """Build + compile the per-core Bass program for capacity C (mult of 128).

    All inputs are shipped pre-arranged in SBUF partition-major layout so
    every DMA is ~128 large contiguous descriptors."""
    assert C % 128 == 0
    nsub = C // 128  # token subtiles
    KA = D // 128  # 8 contraction chunks for matmul 1
    KM = H // 128  # 16 contraction chunks for matmul 2
    blocks = _blocks_for(C)
    NB = len(blocks)

    nc = bacc.Bacc("TRN2", target_bir_lowering=False, debug=False)

    # x per token block, partition-major, contiguous per partition.  The
    # first matmuls' gating inputs ship as two "bundles", each one flat
    # contiguous transfer (descriptor generation costs ~730ns per dma_start
    # on the sequencer, so the gating set uses as few transfers as
    # possible): gb1 = [x0 half a<4 | w1 piece 0], gb2 = [x0 half a>=4 |
    # w3 piece 0], flattened per partition.
    assert PIECES[0] == 1
    tb0 = blocks[0][1]
    xs0 = KA // 2
    G1X = xs0 * tb0  # x-half width inside a bundle
    G1W = G1X + KA * 128
    gb1 = nc.declare_dram_parameter("gb1", [128, G1W], BF16, isOutput=False)
    gb2 = nc.declare_dram_parameter("gb2", [128, G1W], BF16, isOutput=False)
    xts_rest = [
        nc.declare_dram_parameter(f"xt{b}", [128, KA, tb], BF16, isOutput=False)
        for b, (t0, tb) in enumerate(blocks)
        if b > 0
    ]
    w1ps = {
        p: nc.declare_dram_parameter(
            f"w1p{p}", [128, KA, sz * 128], BF16, isOutput=False
        )
        for p, sz in enumerate(PIECES)
        if p > 0
    }
    w3ps = {
        p: nc.declare_dram_parameter(
            f"w3p{p}", [128, KA, sz * 128], BF16, isOutput=False
        )
        for p, sz in enumerate(PIECES)
        if p > 0
    }
    w2ps = [
        [
            nc.declare_dram_parameter(
                f"w2p{mh}{dh}", [128, KM // 2, 512], BF16, isOutput=False
            )
            for dh in range(2)
        ]
        for mh in range(2)
    ]
    cwt = nc.declare_dram_parameter("cwt", [128, nsub], F32, isOutput=False)
    y = nc.declare_dram_parameter("y", [C, D], F32, isOutput=True)
    y_t = y.rearrange("(n p) d -> p n d", p=128)  # [128, nsub, D]

    with ExitStack() as ctx:
        tc = ctx.enter_context(tile.TileContext(nc))
        wpool = ctx.enter_context(tc.tile_pool(name="weights", bufs=1))
        xpool = ctx.enter_context(tc.tile_pool(name="x", bufs=3))
        hpool = ctx.enter_context(tc.tile_pool(name="h", bufs=2))
        spool = ctx.enter_context(tc.tile_pool(name="s", bufs=3))
        ypool = ctx.enter_context(tc.tile_pool(name="y", bufs=2))
        ppool = ctx.enter_context(tc.tile_pool(name="psum", bufs=2, space="PSUM"))

        # HAM warmup: dummy matmuls that depend on nothing but two memsets,
        # so they run during the NEFF preamble + gating DMA wait and the PE
        # activity window is gap-free from first issue.
        # One shared psum tile: dummies order by PE program order alone (a
        # fresh tile per dummy adds cross-tile WAW semaphore round-trips
        # that space the dummies ~0.5us apart).
        warmw = wpool.tile([128, 128], BF16, tag="warmw")
        warmx = wpool.tile([128, 512], BF16, tag="warmx")
        nc.vector.memset(warmw[:], 0)
        nc.vector.memset(warmx[:], 0)
        wp = ppool.tile([128, 512], F32, tag="ph1")
        for _ in range(WARMUP_MM):
            nc.tensor.matmul(wp[:], warmw[:], warmx[:], start=True, stop=True)

        # Gating transfers in consumption order, all on the sync ring: a
        # single ring's FIFO gives the early transfers strict priority.
        # (Splitting them across rings was measured slower: the HW queues
        # round-robin between rings, so late bytes interleave ahead of the
        # gating bytes.)
        gbt1 = wpool.tile([128, G1W], BF16, tag="gb1")
        nc.sync.dma_start(gbt1[:], gb1[:])
        gbt2 = wpool.tile([128, G1W], BF16, tag="gb2")
        nc.sync.dma_start(gbt2[:], gb2[:])

        def xacc0(a):
            g = gbt1 if a < xs0 else gbt2
            return g[:, (a % xs0) * tb0 : (a % xs0 + 1) * tb0]

        xacc = {0: xacc0}
        if NB > 1:
            tb1 = blocks[1][1]
            xts1 = xpool.tile([128, KA, tb1], BF16, tag="xts")
            nc.sync.dma_start(xts1[:], xts_rest[0][:])
            xacc[1] = lambda a: xts1[:, a, :]

        # per m-chunk weight accessors: a -> [128, 128] stationary slice
        w1p = [lambda a: gbt1[:, G1X + a * 128 : G1X + (a + 1) * 128]]
        w3p = [lambda a: gbt2[:, G1X + a * 128 : G1X + (a + 1) * 128]]

        def _wsl(t, i):
            return lambda a: t[:, a, bass.ts(i, 128)]

        for p, sz in enumerate(PIECES):
            if p == 0:
                continue
            t1 = wpool.tile([128, KA, sz * 128], BF16, tag=f"w1s{p}")
            nc.sync.dma_start(t1[:], w1ps[p][:])
            t3 = wpool.tile([128, KA, sz * 128], BF16, tag=f"w3s{p}")
            nc.sync.dma_start(t3[:], w3ps[p][:])
            for i in range(sz):
                w1p.append(_wsl(t1, i))
                w3p.append(_wsl(t3, i))

        w2p = []  # [m-half][d-half] tiles of [128, KM//2, 512]
        for mh in range(2):
            row = []
            for dh in range(2):
                t2 = wpool.tile([128, KM // 2, 512], BF16, tag=f"w2s{mh}{dh}")
                nc.sync.dma_start(t2[:], w2ps[mh][dh][:])
                row.append(t2)
            w2p.append(row)
        cws = wpool.tile([128, nsub], F32, tag="cws")
        nc.sync.dma_start(cws[:], cwt[:])

        def phase_a(group):
            """Phase A for a group of blocks, m-chunk interleaved across the
            group so each weight piece serves all the group's tokens before
            the next piece is needed."""
            xf_g, hts_g = [], []
            for bi in group:
                t0, tb = blocks[bi]
                if bi in xacc:
                    xf = xacc[bi]
                else:
                    xts = xpool.tile(
                        [128, KA, tb], BF16, tag="xts", name=f"xts{bi}"
                    )
                    nc.sync.dma_start(xts[:], xts_rest[bi - 1][:])
                    xf = lambda a, xts=xts: xts[:, a, :]
                xf_g.append(xf)
                hts_g.append(
                    hpool.tile([128, KM, tb], BF16, tag="hts", name=f"hts{bi}")
                )
            for m in range(KM):
                for gi, bi in enumerate(group):
                    tb = blocks[bi][1]
                    xf, hts = xf_g[gi], hts_g[gi]
                    ph1 = ppool.tile([128, tb], F32, tag="ph1")
                    for a in range(KA):
                        nc.tensor.matmul(
                            ph1[:],
                            w1p[m](a),
                            xf(a),
                            start=(a == 0),
                            stop=(a == KA - 1),
                        )
                    ph3 = ppool.tile([128, tb], F32, tag="ph3")
                    for a in range(KA):
                        nc.tensor.matmul(
                            ph3[:],
                            w3p[m](a),
                            xf(a),
                            start=(a == 0),
                            stop=(a == KA - 1),
                        )
                    sil = spool.tile([128, tb], BF16, tag="sil")
                    nc.scalar.activation(
                        sil[:], ph1[:], mybir.ActivationFunctionType.Silu
                    )
                    nc.vector.tensor_mul(hts[:, m, :], sil[:], ph3[:])
            return hts_g

        def phase_b(bi, hts):
            # y = hT.T @ w2T, scaled by cw.  In the final block each subtile
            # is stored as soon as it is scaled; the final subtile is split
            # into narrowing pieces so earlier pieces' scale+store overlap
            # the remaining matmuls and only one small store (plus its HBM
            # write receipt) trails the last matmul.
            t0, tb = blocks[bi]
            nsub_b = tb // 128
            gn0 = t0 // 128
            last_block = bi == NB - 1
            ysb = ypool.tile([128, nsub_b, 1024], F32, tag="ysb")
            for n in range(nsub_b):
                nsl = bass.ts(n, 128)
                gn = gn0 + n  # global subtile index
                final_sub = last_block and n == nsub_b - 1
                if not final_sub:
                    py0 = ppool.tile([128, 512], F32, tag="py0")
                    py1 = ppool.tile([128, 512], F32, tag="py1")
                    for m in range(KM):
                        mh, mr = divmod(m, KM // 2)
                        nc.tensor.matmul(
                            py0[:],
                            hts[:, m, nsl],
                            w2p[mh][0][:, mr, :],
                            start=(m == 0),
                            stop=(m == KM - 1),
                        )
                        nc.tensor.matmul(
                            py1[:],
                            hts[:, m, nsl],
                            w2p[mh][1][:, mr, :],
                            start=(m == 0),
                            stop=(m == KM - 1),
                        )
                    nc.vector.tensor_scalar_mul(
                        ysb[:, n, 0:512], py0[:], cws[:, gn : gn + 1]
                    )
                    nc.vector.tensor_scalar_mul(
                        ysb[:, n, 512:1024], py1[:], cws[:, gn : gn + 1]
                    )
                    if last_block:
                        nc.scalar.dma_start(y_t[:, gn, :], ysb[:, n, :])
                else:
                    pieces = [(0, 0, 256), (0, 256, 256), (1, 0, 256),
                              (1, 256, 128), (1, 384, 128)]
                    for q, (dh, off, wd) in enumerate(pieces):
                        py = ppool.tile([128, wd], F32, tag=f"py{q % 2}")
                        qsl = slice(off, off + wd)
                        for m in range(KM):
                            mh, mr = divmod(m, KM // 2)
                            nc.tensor.matmul(
                                py[:],
                                hts[:, m, nsl],
                                w2p[mh][dh][:, mr, qsl],
                                start=(m == 0),
                                stop=(m == KM - 1),
                            )
                        dsl = slice(dh * 512 + off, dh * 512 + off + wd)
                        nc.vector.tensor_scalar_mul(
                            ysb[:, n, dsl], py[:], cws[:, gn : gn + 1]
                        )
                        nc.scalar.dma_start(y_t[:, gn, dsl], ysb[:, n, dsl])
            if not last_block:
                nc.scalar.dma_start(
                    y_t[:, gn0 : gn0 + nsub_b, :], ysb[:]
                )

        # Software pipeline: A{0,1} interleaved, then B0, A2, B1, A3, B2,
        # ..., B_{NB-1}.  Phase B stays a full block behind phase A so
        # block 0's phase B (first reader of the late-loaded w2) starts
        # ~60us in, after w2 lands; hpool bufs=2 holds the two live hts.
        groups = [[0]] if NB == 1 else [[0, 1]] + [[b] for b in range(2, NB)]
        hts_live = {}
        for bi, hts in zip(groups[0], phase_a(groups[0])):
            hts_live[bi] = hts
        b_next = 0
        for g in groups[1:]:
            phase_b(b_next, hts_live.pop(b_next))
            b_next += 1
            for bi, hts in zip(g, phase_a(g)):
                hts_live[bi] = hts
        while b_next < NB:
            phase_b(b_next, hts_live.pop(b_next))
            b_next += 1

    nc.compile()
    return nc


def _get(C: int):
    if C not in _CACHE:
        _CACHE[C] = _build(C)
    return _CACHE[C]


def _prepare_core_inputs(x2d, w1, w2, w3, rows, cw_e, C):
    bf = ml_dtypes.bfloat16
    nsub = C // 128
    KA, KM = D // 128, H // 128
    blocks = _blocks_for(C)
    xs0 = KA // 2
    in_maps = []
    for e in range(E):
        ce = len(rows[e])
        xt = np.zeros((D, C), bf)
        xt[:, :ce] = x2d[rows[e]].T.astype(bf)
        # partition-major: [128, KA, C]
        xpm = np.ascontiguousarray(xt.reshape(KA, 128, C).transpose(1, 0, 2))

        w1pm = w1[e].T.astype(bf).reshape(KA, 128, H).transpose(1, 0, 2)
        w3pm = w3[e].T.astype(bf).reshape(KA, 128, H).transpose(1, 0, 2)
        w2pm = w2[e].T.astype(bf).reshape(KM, 128, D).transpose(1, 0, 2)

        cwt = np.zeros((C,), np.float32)
        cwt[:ce] = cw_e[e]

        m = {"cwt": np.ascontiguousarray(cwt.reshape(nsub, 128).T)}
        t0_0, tb0 = blocks[0]
        G1X = xs0 * tb0
        gb = np.empty((2, 128, G1X + KA * 128), bf)
        gb[0, :, :G1X] = xpm[:, 0:xs0, 0:tb0].reshape(128, G1X)
        gb[1, :, :G1X] = xpm[:, xs0:KA, 0:tb0].reshape(128, G1X)
        gb[0, :, G1X:] = w1pm[:, :, 0:128].reshape(128, KA * 128)
        gb[1, :, G1X:] = w3pm[:, :, 0:128].reshape(128, KA * 128)
        m["gb1"] = np.ascontiguousarray(gb[0])
        m["gb2"] = np.ascontiguousarray(gb[1])
        for b, (t0, tb) in enumerate(blocks):
            if b == 0:
                continue
            m[f"xt{b}"] = np.ascontiguousarray(xpm[:, :, t0 : t0 + tb])
        m0 = 0
        for p, sz in enumerate(PIECES):
            if p > 0:
                hs = slice(m0 * 128, (m0 + sz) * 128)
                m[f"w1p{p}"] = np.ascontiguousarray(w1pm[:, :, hs])
                m[f"w3p{p}"] = np.ascontiguousarray(w3pm[:, :, hs])
            m0 += sz
        for mh in range(2):
            msl = slice(mh * (KM // 2), (mh + 1) * (KM // 2))
            for dh in range(2):
                m[f"w2p{mh}{dh}"] = np.ascontiguousarray(
                    w2pm[:, msl, dh * 512 : (dh + 1) * 512]
                )
        in_maps.append(m)
    return in_maps


def run(inputs: dict, trace: bool = False, trace_cores=None):
    """Core implementation; returns (output, BassKernelResults)."""
    x = np.asarray(inputs["x"])
    router_w = np.asarray(inputs["router_w"], np.float32)
    w1 = np.asarray(inputs["w1"], np.float32)
    w2 = np.asarray(inputs["w2"], np.float32)
    w3 = np.asarray(inputs["w3"], np.float32)

    B, S, _ = x.shape
    assert x.shape[-1] == D and router_w.shape == (E, D), (x.shape, router_w.shape)
    assert w1.shape == (E, H, D) and w3.shape == (E, H, D) and w2.shape == (E, D, H)
    x2d = np.ascontiguousarray(x.reshape(-1, D).astype(np.float32))
    T = x2d.shape[0]

    rows, cw_e, top2, slot = _route(x2d, router_w)
    rows_d = [r[:CAP] for r in rows]
    cw_d = [c[:CAP] for c in cw_e]
    spill = [
        (e, rows[e][CAP:], cw_e[e][CAP:]) for e in range(E) if len(rows[e]) > CAP
    ]
    cmax = max(len(r) for r in rows_d)
    C = max(128, int(np.ceil(cmax / 128) * 128))

    nc = _get(C)
    in_maps = _prepare_core_inputs(x2d, w1, w2, w3, rows_d, cw_d, C)
    res = run_bass_kernel_spmd(
        nc,
        in_maps,
        list(range(E)),
        trace=trace,
        trace_cores=trace_cores,
    )

    Y = np.stack([res.results[e]["y"] for e in range(E)])  # [E, C, D] f32
    Yf = Y.reshape(E * C, D)
    valid = slot < C  # [T, 2]; spilled pairs resolved on host below
    fi = top2.astype(np.int64) * C + np.minimum(slot, C - 1)
    out = Yf[fi[:, 0]] * valid[:, 0:1] + Yf[fi[:, 1]] * valid[:, 1:2]

    for e, r, c in spill:
        xo = x2d[r]
        h1 = xo @ w1[e].T
        h = (h1 / (1.0 + np.exp(-h1))) * (xo @ w3[e].T)
        out[r] += (h @ w2[e].T) * c[:, None]
    return out.reshape(B, S, D).astype(x.dtype), res


def kernel(**inputs) -> np.ndarray:
    out, _ = run(inputs, trace=False)
    return out


# revision 23
# speedup vs baseline: 1.0098x; 1.0012x over previous
"""Mixture-of-Experts (8 experts, top-2, D=1024, H=2048, T=8192) on 8 trn2 cores.

Strategy: expert-parallel with host-side routing and capacity factor 1.0.
  - Router (tiny: [T,D]@[D,E]) runs on host in float64; top-2 selection was
    verified to match fp32 jax (cpu + neuron) selection for this problem size.
  - Each core owns one expert and computes SwiGLU on the tokens routed to it,
    capped at CAP = T*TOP_K/E = 2048 tokens (capacity factor 1.0).  Spillover
    pairs beyond the cap (<1% of pairs for near-balanced routing) are computed
    exactly on the host during the combine.  Without the cap, SPMD would force
    every core to the hottest expert's padded count (17 x 128-token subtiles
    instead of 16, ~6% more tensor work on every core).
  - Activations flow in transposed (feature-major) layout so the kernel needs
    no on-device transposes:
        h1T = w1 @ xT   (accumulate over D chunks)   [H, C]
        hT  = silu(h1T) * h3T                        [H, C]  (bf16)
        y   = (hT.T chunks) @ w2T                    [C, D]  (tokens on
              partitions so the per-token combine-weight scale is a
              per-partition tensor_scalar op)
  - Host combines: out[t] = y_e1[slot1] + y_e2[slot2] (cw applied on device),
    plus the exact spillover contributions.

Schedule notes (from perfetto traces; the PE clock is bimodal run-to-run,
~2.0 or ~2.4 GHz depending on package power state):
  - Token blocks are [256, 256, 512, ..., 512].  The two small first
    blocks run phase A interleaved at m-chunk granularity, so the gating
    DMA set stays small while the weight-piece appetite stays at
    ~146 GB/s (a solo 256-token block would starve on HBM at 2.4 GHz).
  - Phase order is software-pipelined A{0,1}, B0, A2, B1, ... so block 0's
    phase B (which reads all of w2, 4 MB, loaded last) runs ~65us in,
    after w2 lands.  hpool bufs=2 holds the two live hts tiles.
  - A dma_start costs ~730ns of DIRECT2D descriptor generation on the
    issuing sequencer, so the first matmuls' inputs ship as two flat
    bundles gb1=[x0 a<4 | w1 piece0], gb2=[x0 a>=4 | w3 piece0] — one
    transfer each, accessed by flat column slices.  All input loads stay
    on the single sync ring: the HW queues round-robin between rings, so
    spreading loads across rings lets late bytes overtake gating bytes
    (measured slower).  Every transfer is contiguous per partition.
  - The PE starts at half duty (HAM k=4/8) and is promoted after ~3.4us of
    gap-free matmul activity; WARMUP_MM dummy matmuls into one shared psum
    tile bridge from first possible issue (~8.1us, after the NEFF
    preamble) to the gating bundle receipt (~11-12us).
  - In the final block every non-final subtile is stored as soon as it is
    scaled, and the final subtile is stored in narrowing pieces
    (256/256/256/128/128 wide) so only one small store (plus its HBM write
    receipt) trails the last matmul.
  - Not worth it / dead ends (measured): fp8 (e4m3 on even one matmul
    operand gives 3.3% rel err vs the 2e-2 gate); splitting loads or the
    tail stores across DMA rings; a ~150ns matmul hiccup every 12.95us of
    wall time is external (power management) and unfixable.
"""

import sys
import types
from contextlib import ExitStack

import ml_dtypes
import numpy as np

import concourse.bass as bass
import concourse.tile as tile
from concourse import bacc, mybir
from concourse.bass_utils import run_bass_kernel_spmd


def install_axon_hooks_shim():
    """The container's antenv stub lacks axon_hooks, which
    run_bass_kernel_spmd imports whenever tracing is requested (including
    via the BASS_TRACE env var). Recreate it and register the NTFF
    profiling hook if the axon PJRT .so is present."""
    try:
        import antenv
    except ImportError:
        return False
    if "antenv.axon_hooks" in sys.modules:
        return sys.modules["antenv.axon_hooks"]._hook is not None
    mod = types.ModuleType("antenv.axon_hooks")
    mod._hook = None
    mod.set_axon_ntff_profile_hook = lambda h: setattr(mod, "_hook", h)
    mod.get_axon_ntff_profile_hook = lambda: mod._hook
    sys.modules["antenv.axon_hooks"] = mod
    antenv.axon_hooks = mod
    try:
        from trn_agent_boot.trn_boot import _ntff_profile_via_ctypes

        mod.set_axon_ntff_profile_hook(
            _ntff_profile_via_ctypes("/opt/axon/libaxon_pjrt.so")
        )
    except Exception:
        pass
    return mod._hook is not None


install_axon_hooks_shim()

E = 8  # experts == cores
D = 1024
H = 2048
TOP_K = 2

# Device capacity per expert (capacity factor 1.0: T * TOP_K / E tokens).
# Spillover pairs beyond this (<1% of pairs for near-balanced routing) are
# computed exactly on the host during the combine, so every core runs the
# same balanced 16-subtile program instead of padding all cores to the
# hottest expert's count.
CAP = 2048

BF16 = mybir.dt.bfloat16
F32 = mybir.dt.float32

_CACHE: dict[int, object] = {}


def _route(x2d: np.ndarray, router_w: np.ndarray):
    """Float64 router. Returns per-expert token lists, per-expert combine
    weights, and for each token its (expert, slot-in-expert-batch) pairs."""
    T = x2d.shape[0]
    logits = x2d.astype(np.float64) @ router_w.astype(np.float64).T  # [T, E]
    order = np.argsort(-logits, axis=1, kind="stable")
    top2 = order[:, :TOP_K]  # [T, 2]
    lt = np.take_along_axis(logits, top2, axis=1)
    m = lt.max(axis=1, keepdims=True)
    ex = np.exp(lt - m)
    cw = (ex / ex.sum(axis=1, keepdims=True)).astype(np.float32)  # [T, 2]

    rows = []  # rows[e]: token ids routed to expert e (ascending)
    cw_e = []  # cw_e[e]: combine weight per routed token
    slot = np.empty((T, TOP_K), np.int64)  # slot[t, k]: row of t in expert batch
    for e in range(E):
        r = np.where((top2[:, 0] == e) | (top2[:, 1] == e))[0]
        k = np.where(top2[r, 0] == e, 0, 1)
        rows.append(r)
        cw_e.append(cw[r, k])
        slot[r, k] = np.arange(len(r))
    return rows, cw_e, top2, slot


# w1/w3 piece sizes in m-chunks (small first so early matmuls start early)
PIECES = (1, 1, 2, 4, 4, 4)

# dummy matmuls issued before the first DMA-gated matmul: bridge from first
# possible PE issue (~8.3us, after the NEFF preamble) to the gating DMA
# receipt (~10.5-11us: barrier 7.2 + 2 DIRECT2D descgens + ~1.4us transfer),
# keeping the HAM activity window gap-free so promotion lands ~3.4us after
# first issue instead of ~3.4us after the gating receipt.
WARMUP_MM = 7


def _blocks_for(C):
    """Token blocks: [256, 256, 512, ..., 512, rem] (rem <= 512).  The two
    small first blocks run phase A interleaved at m-chunk granularity: the
    gating DMA set for the first matmul stays small (x half + w1 piece 0 =
    512 KB) while the weight-piece appetite stays at ~146 GB/s (a solo
    256-token block would consume pieces at ~293 GB/s and starve on HBM)."""
    if C <= 512:
        return [(0, C)]
    blocks = [(0, 256), (256, 256)]
    t0 = 512
    while C - t0 > 512:
        blocks.append((t0, 512))
        t0 += 512
    if C > t0:
        blocks.append((t0, C - t0))
    return blocks


def _build(C: int):
    """Build + compile the per-core Bass program for capacity C (mult of 128).

    All inputs are shipped pre-arranged in SBUF partition-major layout so
    every DMA is ~128 large contiguous descriptors."""
    assert C % 128 == 0
    nsub = C // 128  # token subtiles
    KA = D // 128  # 8 contraction chunks for matmul 1
    KM = H // 128  # 16 contraction chunks for matmul 2
    blocks = _blocks_for(C)
    NB = len(blocks)

    nc = bacc.Bacc("TRN2", target_bir_lowering=False, debug=False)

    # x per token block, partition-major, contiguous per partition.  The
    # first matmuls' gating inputs ship as two "bundles", each one flat
    # contiguous transfer (descriptor generation costs ~730ns per dma_start
    # on the sequencer, so the gating set uses as few transfers as
    # possible): gb1 = [x0 half a<4 | w1 piece 0], gb2 = [x0 half a>=4 |
    # w3 piece 0], flattened per partition.
    assert PIECES[0] == 1
    tb0 = blocks[0][1]
    xs0 = KA // 2
    G1X = xs0 * tb0  # x-half width inside a bundle
    G1W = G1X + KA * 128
    gb1 = nc.declare_dram_parameter("gb1", [128, G1W], BF16, isOutput=False)
    gb2 = nc.declare_dram_parameter("gb2", [128, G1W], BF16, isOutput=False)
    xts_rest = [
        nc.declare_dram_parameter(f"xt{b}", [128, KA, tb], BF16, isOutput=False)
        for b, (t0, tb) in enumerate(blocks)
        if b > 0
    ]
    w1ps = {
        p: nc.declare_dram_parameter(
            f"w1p{p}", [128, KA, sz * 128], BF16, isOutput=False
        )
        for p, sz in enumerate(PIECES)
        if p > 0
    }
    w3ps = {
        p: nc.declare_dram_parameter(
            f"w3p{p}", [128, KA, sz * 128], BF16, isOutput=False
        )
        for p, sz in enumerate(PIECES)
        if p > 0
    }
    w2ps = [
        [
            nc.declare_dram_parameter(
                f"w2p{mh}{dh}", [128, KM // 2, 512], BF16, isOutput=False
            )
            for dh in range(2)
        ]
        for mh in range(2)
    ]
    cwt = nc.declare_dram_parameter("cwt", [128, nsub], F32, isOutput=False)
    y = nc.declare_dram_parameter("y", [C, D], F32, isOutput=True)
    y_t = y.rearrange("(n p) d -> p n d", p=128)  # [128, nsub, D]

    with ExitStack() as ctx:
        tc = ctx.enter_context(tile.TileContext(nc))
        wpool = ctx.enter_context(tc.tile_pool(name="weights", bufs=1))
        xpool = ctx.enter_context(tc.tile_pool(name="x", bufs=3))
        hpool = ctx.enter_context(tc.tile_pool(name="h", bufs=2))
        spool = ctx.enter_context(tc.tile_pool(name="s", bufs=3))
        ypool = ctx.enter_context(tc.tile_pool(name="y", bufs=2))
        ppool = ctx.enter_context(tc.tile_pool(name="psum", bufs=2, space="PSUM"))

        # HAM warmup: dummy matmuls that depend on nothing but two memsets,
        # so they run during the NEFF preamble + gating DMA wait and the PE
        # activity window is gap-free from first issue.
        # One shared psum tile: dummies order by PE program order alone (a
        # fresh tile per dummy adds cross-tile WAW semaphore round-trips
        # that space the dummies ~0.5us apart).
        warmw = wpool.tile([128, 128], BF16, tag="warmw")
        warmx = wpool.tile([128, 512], BF16, tag="warmx")
        nc.vector.memset(warmw[:], 0)
        nc.vector.memset(warmx[:], 0)
        wp = ppool.tile([128, 512], F32, tag="ph1")
        for _ in range(WARMUP_MM):
            nc.tensor.matmul(wp[:], warmw[:], warmx[:], start=True, stop=True)

        # Gating transfers in consumption order, all on the sync ring: a
        # single ring's FIFO gives the early transfers strict priority.
        # (Splitting them across rings was measured slower: the HW queues
        # round-robin between rings, so late bytes interleave ahead of the
        # gating bytes.)
        gbt1 = wpool.tile([128, G1W], BF16, tag="gb1")
        nc.sync.dma_start(gbt1[:], gb1[:])
        gbt2 = wpool.tile([128, G1W], BF16, tag="gb2")
        nc.sync.dma_start(gbt2[:], gb2[:])

        def xacc0(a):
            g = gbt1 if a < xs0 else gbt2
            return g[:, (a % xs0) * tb0 : (a % xs0 + 1) * tb0]

        xacc = {0: xacc0}
        if NB > 1:
            tb1 = blocks[1][1]
            xts1 = xpool.tile([128, KA, tb1], BF16, tag="xts")
            nc.sync.dma_start(xts1[:], xts_rest[0][:])
            xacc[1] = lambda a: xts1[:, a, :]

        # per m-chunk weight accessors: a -> [128, 128] stationary slice
        w1p = [lambda a: gbt1[:, G1X + a * 128 : G1X + (a + 1) * 128]]
        w3p = [lambda a: gbt2[:, G1X + a * 128 : G1X + (a + 1) * 128]]

        def _wsl(t, i):
            return lambda a: t[:, a, bass.ts(i, 128)]

        for p, sz in enumerate(PIECES):
            if p == 0:
                continue
            t1 = wpool.tile([128, KA, sz * 128], BF16, tag=f"w1s{p}")
            nc.sync.dma_start(t1[:], w1ps[p][:])
            t3 = wpool.tile([128, KA, sz * 128], BF16, tag=f"w3s{p}")
            nc.sync.dma_start(t3[:], w3ps[p][:])
            for i in range(sz):
                w1p.append(_wsl(t1, i))
                w3p.append(_wsl(t3, i))

        w2p = []  # [m-half][d-half] tiles of [128, KM//2, 512]
        for mh in range(2):
            row = []
            for dh in range(2):
                t2 = wpool.tile([128, KM // 2, 512], BF16, tag=f"w2s{mh}{dh}")
                nc.sync.dma_start(t2[:], w2ps[mh][dh][:])
                row.append(t2)
            w2p.append(row)
        cws = wpool.tile([128, nsub], F32, tag="cws")
        nc.sync.dma_start(cws[:], cwt[:])

        def phase_a(group):
            """Phase A for a group of blocks, m-chunk interleaved across the
            group so each weight piece serves all the group's tokens before
            the next piece is needed."""
            xf_g, hts_g = [], []
            for bi in group:
                t0, tb = blocks[bi]
                if bi in xacc:
                    xf = xacc[bi]
                else:
                    xts = xpool.tile(
                        [128, KA, tb], BF16, tag="xts", name=f"xts{bi}"
                    )
                    nc.sync.dma_start(xts[:], xts_rest[bi - 1][:])
                    xf = lambda a, xts=xts: xts[:, a, :]
                xf_g.append(xf)
                hts_g.append(
                    hpool.tile([128, KM, tb], BF16, tag="hts", name=f"hts{bi}")
                )
            for m in range(KM):
                for gi, bi in enumerate(group):
                    tb = blocks[bi][1]
                    xf, hts = xf_g[gi], hts_g[gi]
                    ph1 = ppool.tile([128, tb], F32, tag="ph1")
                    for a in range(KA):
                        nc.tensor.matmul(
                            ph1[:],
                            w1p[m](a),
                            xf(a),
                            start=(a == 0),
                            stop=(a == KA - 1),
                        )
                    ph3 = ppool.tile([128, tb], F32, tag="ph3")
                    for a in range(KA):
                        nc.tensor.matmul(
                            ph3[:],
                            w3p[m](a),
                            xf(a),
                            start=(a == 0),
                            stop=(a == KA - 1),
                        )
                    sil = spool.tile([128, tb], BF16, tag="sil")
                    nc.scalar.activation(
                        sil[:], ph1[:], mybir.ActivationFunctionType.Silu
                    )
                    nc.vector.tensor_mul(hts[:, m, :], sil[:], ph3[:])
            return hts_g

        def phase_b(bi, hts):
            # y = hT.T @ w2T, scaled by cw.  In the final block each subtile
            # is stored as soon as it is scaled; the final subtile is split
            # into narrowing pieces so earlier pieces' scale+store overlap
            # the remaining matmuls and only one small store (plus its HBM
            # write receipt) trails the last matmul.
            t0, tb = blocks[bi]
            nsub_b = tb // 128
            gn0 = t0 // 128
            last_block = bi == NB - 1
            ysb = ypool.tile([128, nsub_b, 1024], F32, tag="ysb")
            for n in range(nsub_b):
                nsl = bass.ts(n, 128)
                gn = gn0 + n  # global subtile index
                final_sub = last_block and n == nsub_b - 1
                if not final_sub:
                    py0 = ppool.tile([128, 512], F32, tag="py0")
                    py1 = ppool.tile([128, 512], F32, tag="py1")
                    for m in range(KM):
                        mh, mr = divmod(m, KM // 2)
                        nc.tensor.matmul(
                            py0[:],
                            hts[:, m, nsl],
                            w2p[mh][0][:, mr, :],
                            start=(m == 0),
                            stop=(m == KM - 1),
                        )
                        nc.tensor.matmul(
                            py1[:],
                            hts[:, m, nsl],
                            w2p[mh][1][:, mr, :],
                            start=(m == 0),
                            stop=(m == KM - 1),
                        )
                    nc.vector.tensor_scalar_mul(
                        ysb[:, n, 0:512], py0[:], cws[:, gn : gn + 1]
                    )
                    nc.vector.tensor_scalar_mul(
                        ysb[:, n, 512:1024], py1[:], cws[:, gn : gn + 1]
                    )
                    if last_block:
                        nc.scalar.dma_start(y_t[:, gn, :], ysb[:, n, :])
                else:
                    pieces = [(0, 0, 256), (0, 256, 256), (1, 0, 256),
                              (1, 256, 128), (1, 384, 128)]
                    for q, (dh, off, wd) in enumerate(pieces):
                        py = ppool.tile([128, wd], F32, tag=f"py{q % 2}")
                        qsl = slice(off, off + wd)
                        for m in range(KM):
                            mh, mr = divmod(m, KM // 2)
                            nc.tensor.matmul(
                                py[:],
                                hts[:, m, nsl],
                                w2p[mh][dh][:, mr, qsl],
                                start=(m == 0),
                                stop=(m == KM - 1),
                            )
                        dsl = slice(dh * 512 + off, dh * 512 + off + wd)
                        nc.vector.tensor_scalar_mul(
                            ysb[:, n, dsl], py[:], cws[:, gn : gn + 1]
                        )
                        nc.scalar.dma_start(y_t[:, gn, dsl], ysb[:, n, dsl])
            if not last_block:
                nc.scalar.dma_start(
                    y_t[:, gn0 : gn0 + nsub_b, :], ysb[:]
                )

        # Software pipeline: A{0,1} interleaved, then B0, A2, B1, A3, B2,
        # ..., B_{NB-1}.  Phase B stays a full block behind phase A so
        # block 0's phase B (first reader of the late-loaded w2) starts
        # ~60us in, after w2 lands; hpool bufs=2 holds the two live hts.
        groups = [[0]] if NB == 1 else [[0, 1]] + [[b] for b in range(2, NB)]
        hts_live = {}
        for bi, hts in zip(groups[0], phase_a(groups[0])):
            hts_live[bi] = hts
        b_next = 0
        for g in groups[1:]:
            phase_b(b_next, hts_live.pop(b_next))
            b_next += 1
            for bi, hts in zip(g, phase_a(g)):
                hts_live[bi] = hts
        while b_next < NB:
            phase_b(b_next, hts_live.pop(b_next))
            b_next += 1

    nc.compile()
    return nc


def _get(C: int):
    if C not in _CACHE:
        _CACHE[C] = _build(C)
    return _CACHE[C]


def _prepare_core_inputs(x2d, w1, w2, w3, rows, cw_e, C):
    bf = ml_dtypes.bfloat16
    nsub = C // 128
    KA, KM = D // 128, H // 128
    blocks = _blocks_for(C)
    xs0 = KA // 2
    in_maps = []
    for e in range(E):
        ce = len(rows[e])
        xt = np.zeros((D, C), bf)
        xt[:, :ce] = x2d[rows[e]].T.astype(bf)
        # partition-major: [128, KA, C]
        xpm = np.ascontiguousarray(xt.reshape(KA, 128, C).transpose(1, 0, 2))

        w1pm = w1[e].T.astype(bf).reshape(KA, 128, H).transpose(1, 0, 2)
        w3pm = w3[e].T.astype(bf).reshape(KA, 128, H).transpose(1, 0, 2)
        w2pm = w2[e].T.astype(bf).reshape(KM, 128, D).transpose(1, 0, 2)

        cwt = np.zeros((C,), np.float32)
        cwt[:ce] = cw_e[e]

        m = {"cwt": np.ascontiguousarray(cwt.reshape(nsub, 128).T)}
        t0_0, tb0 = blocks[0]
        G1X = xs0 * tb0
        gb = np.empty((2, 128, G1X + KA * 128), bf)
        gb[0, :, :G1X] = xpm[:, 0:xs0, 0:tb0].reshape(128, G1X)
        gb[1, :, :G1X] = xpm[:, xs0:KA, 0:tb0].reshape(128, G1X)
        gb[0, :, G1X:] = w1pm[:, :, 0:128].reshape(128, KA * 128)
        gb[1, :, G1X:] = w3pm[:, :, 0:128].reshape(128, KA * 128)
        m["gb1"] = np.ascontiguousarray(gb[0])
        m["gb2"] = np.ascontiguousarray(gb[1])
        for b, (t0, tb) in enumerate(blocks):
            if b == 0:
                continue
            m[f"xt{b}"] = np.ascontiguousarray(xpm[:, :, t0 : t0 + tb])
        m0 = 0
        for p, sz in enumerate(PIECES):
            if p > 0:
                hs = slice(m0 * 128, (m0 + sz) * 128)
                m[f"w1p{p}"] = np.ascontiguousarray(w1pm[:, :, hs])
                m[f"w3p{p}"] = np.ascontiguousarray(w3pm[:, :, hs])
            m0 += sz
        for mh in range(2):
            msl = slice(mh * (KM // 2), (mh + 1) * (KM // 2))
            for dh in range(2):
                m[f"w2p{mh}{dh}"] = np.ascontiguousarray(
                    w2pm[:, msl, dh * 512 : (dh + 1) * 512]
                )
        in_maps.append(m)
    return in_maps


def run(inputs: dict, trace: bool = False, trace_cores=None):
    """Core implementation; returns (output, BassKernelResults)."""
    x = np.asarray(inputs["x"])
    router_w = np.asarray(inputs["router_w"], np.float32)
    w1 = np.asarray(inputs["w1"], np.float32)
    w2 = np.asarray(inputs["w2"], np.float32)
    w3 = np.asarray(inputs["w3"], np.float32)

    B, S, _ = x.shape
    assert x.shape[-1] == D and router_w.shape == (E, D), (x.shape, router_w.shape)
    assert w1.shape == (E, H, D) and w3.shape == (E, H, D) and w2.shape == (E, D, H)
    x2d = np.ascontiguousarray(x.reshape(-1, D).astype(np.float32))
    T = x2d.shape[0]

    rows, cw_e, top2, slot = _route(x2d, router_w)
    rows_d = [r[:CAP] for r in rows]
    cw_d = [c[:CAP] for c in cw_e]
    spill = [
        (e, rows[e][CAP:], cw_e[e][CAP:]) for e in range(E) if len(rows[e]) > CAP
    ]
    cmax = max(len(r) for r in rows_d)
    C = max(128, int(np.ceil(cmax / 128) * 128))

    nc = _get(C)
    in_maps = _prepare_core_inputs(x2d, w1, w2, w3, rows_d, cw_d, C)
    res = run_bass_kernel_spmd(
        nc,
        in_maps,
        list(range(E)),
        trace=trace,
        trace_cores=trace_cores,
    )

    Y = np.stack([res.results[e]["y"] for e in range(E)])  # [E, C, D] f32
    Yf = Y.reshape(E * C, D)
    valid = slot < C  # [T, 2]; spilled pairs resolved on host below
    fi = top2.astype(np.int64) * C + np.minimum(slot, C - 1)
    out = Yf[fi[:, 0]] * valid[:, 0:1] + Yf[fi[:, 1]] * valid[:, 1:2]

    for e, r, c in spill:
        xo = x2d[r]
        h1 = xo @ w1[e].T
        h = (h1 / (1.0 + np.exp(-h1))) * (xo @ w3[e].T)
        out[r] += (h @ w2[e].T) * c[:, None]
    return out.reshape(B, S, D).astype(x.dtype), res


def kernel(**inputs) -> np.ndarray:
    out, _ = run(inputs, trace=False)
    return out
